# revision 3
# baseline (speedup 1.0000x reference)
"""FM layer (embedding lookup) Trainium2 Bass kernel.

Strategy (data-parallel over batch, 8 cores x 1024 samples):
  - Combined table T [2^20 rows, 128 cols] fp32: cols 0:64 = v_one_hot row,
    col 64 = w_one_hot scalar. 512B rows (dma_gather needs 256B-multiple rows).
  - Host routes each core's 204800 categorical lookups into 256 queues
    (8 sample-blocks x 32 vocab windows of 32768 rows) so the int16-indexed
    dma_gather HW engine can address them (standard embedding index routing).
  - Device: per queue, one dma_gather call pulls the rows into SBUF; the
    per-(row->sample) reduction runs on the TensorEngine as a selection
    matmul (host-built 0/1 fp8 matrices, cast to fp32 on ScalarE), which
    accumulates field sums (and w sums via col 64) into PSUM per sample.
    A second matmul over ACT-squared rows accumulates the squared field sums.
  - Numeric features: small dense matmuls into the same PSUM accumulators.
  - Final FM combine (0.5*(sum fs^2 - sum fss) + first_order + bias) on DVE.
"""
import os
import numpy as np
import ml_dtypes

_SKIP = set(os.environ.get("FMK_SKIP", "").split(","))

import concourse.bass as bass
import concourse.bacc as bacc
import concourse.mybir as mybir
import concourse.tile as tile
from concourse.bass_utils import run_bass_kernel_spmd

B = 8192
F = 200            # categorical fields
NUMER = 100        # numeric features
E = 64             # embedding dim
V = 1_000_000
VPAD = 1_048_576   # 32 windows x 32768
ROW = 128          # table row elems (512B): v(64) + w(1) + pad
NCORES = 8
BC = B // NCORES   # 1024 samples per core
NBLK = BC // 128   # 8 sample blocks per core
NBIN = 32          # vocab windows (32768 rows each)
NCALL = NBLK * NBIN          # 256 gather calls per core
CAP = 1024                   # max lookups per call (avg 800, sigma 28)
NCHUNK = CAP // 128          # 9 sel-matmul chunks per call
LOOKUPS = BC * F             # 204800 per core

_CACHE = {}


def _build_program(trace=False):
    key = "nc"
    if key in _CACHE:
        return _CACHE[key]
    f32 = mybir.dt.float32
    f8 = mybir.dt.float8e4
    nc = bacc.Bacc("TRN2", target_bir_lowering=False, debug=False,
                   num_devices=NCORES)
    t_table = nc.dram_tensor("table", [VPAD, ROW], f32, kind="ExternalInput")
    t_idx = nc.dram_tensor("idxw", [128, NCALL * (CAP // 16)], mybir.dt.int16,
                           kind="ExternalInput")
    t_sel = nc.dram_tensor("sel", [128, NCALL * NCHUNK * 128], f8,
                           kind="ExternalInput")
    t_cnt = nc.dram_tensor("counts", [1, NCALL], mybir.dt.int32,
                           kind="ExternalInput")
    t_numT = nc.dram_tensor("numT", [NUMER, BC], f32, kind="ExternalInput")
    t_vnum = nc.dram_tensor("vnum", [NUMER, E + 1], f32, kind="ExternalInput")
    t_bias = nc.dram_tensor("bias", [128, 1], f32, kind="ExternalInput")
    t_out = nc.dram_tensor("out", [128, NBLK], f32, kind="ExternalOutput")

    cw = CAP // 16  # idx cols per call

    with tile.TileContext(nc) as tc:
        with (
            tc.tile_pool(name="const", bufs=1) as cpool,
            tc.tile_pool(name="gp", bufs=3) as gpool,
            tc.tile_pool(name="rhs", bufs=3) as rhspool,
            tc.tile_pool(name="sel8", bufs=3) as sel8pool,
            tc.tile_pool(name="sel32", bufs=3) as sel32pool,
            tc.tile_pool(name="ps", bufs=2, space="PSUM") as pspool,
            tc.tile_pool(name="small", bufs=2) as smpool,
        ):
            idx_t = cpool.tile([128, NCALL * cw], mybir.dt.int16)
            nc.sync.dma_start(idx_t[:], t_idx[:])
            cnt_t = cpool.tile([1, NCALL], mybir.dt.int32)
            nc.sync.dma_start(cnt_t[:], t_cnt[:])
            numT = cpool.tile([NUMER, BC], f32)
            nc.sync.dma_start(numT[:], t_numT[:])
            vnum = cpool.tile([NUMER, E + 1], f32)
            nc.sync.dma_start(vnum[:], t_vnum[:])
            bias = cpool.tile([128, 1], f32)
            nc.sync.dma_start(bias[:], t_bias[:])

            numT2 = cpool.tile([NUMER, BC], f32)
            vnum2 = cpool.tile([NUMER, E], f32)
            nc.scalar.square(numT2[:], numT[:])
            nc.scalar.square(vnum2[:], vnum[:, 0:E])

            outacc = cpool.tile([128, NBLK], f32)

            cnt_reg = nc.gpsimd.alloc_register("cntreg")

            for b in range(NBLK):
                psum1 = pspool.tile([128, 2 * E + 1], f32, tag="ps1")
                for s in range(NBIN):
                    call = b * NBIN + s
                    g = gpool.tile([128, CAP // 128 * ROW], f32, tag="g")
                    g3 = g[:].rearrange("p (c d) -> p c d", d=ROW)
                    if call < 3:
                        # pad slots are skipped by the gather; on the first
                        # use of each buffer slot, stale NaNs would poison
                        # 0*NaN in the matmul. later calls see old (finite)
                        # gathered data there, which the zero sel rows kill.
                        nc.vector.memset(g[:], 0.0)
                    nc.gpsimd.reg_load(cnt_reg, cnt_t[0:1, call:call + 1])
                    nc.gpsimd.dma_gather(
                        out_ap=g3,
                        in_ap=t_table[s * 32768:(s + 1) * 32768, :],
                        idxs_ap=idx_t[:, call * cw:(call + 1) * cw],
                        num_idxs=CAP,
                        num_idxs_reg=cnt_reg,
                        elem_size=ROW,
                        single_packet=False,
                    )
                    sel8 = sel8pool.tile([128, NCHUNK * 128], f8, tag="sel8")
                    nc.sync.dma_start(
                        sel8[:], t_sel[:, call * NCHUNK * 128:(call + 1) * NCHUNK * 128])
                    sel32 = sel32pool.tile([128, NCHUNK * 128], f32, tag="sel32")
                    nc.scalar.copy(sel32[:], sel8[:])
                    # build rhs [v(64) | w(1) | v^2(64)] = 129 cols per row
                    rhs = rhspool.tile([128, NCHUNK * (2 * E + 1)], f32, tag="rhs")
                    rhs3 = rhs[:].rearrange("p (c d) -> p c d", d=2 * E + 1)
                    nc.vector.tensor_copy(rhs3[:, :, 0:E + 1], g3[:, :, 0:E + 1])
                    nc.scalar.square(rhs3[:, :, E + 1:2 * E + 1], g3[:, :, 0:E])
                    for t in range(NCHUNK):
                        lhsT = sel32[:, t * 128:(t + 1) * 128]
                        nc.tensor.matmul(out=psum1[:], lhsT=lhsT,
                                         rhs=rhs3[:, t, :],
                                         start=(s == 0 and t == 0), stop=False)
                # numeric contribution accumulates on top
                nc.tensor.matmul(out=psum1[:, 0:E + 1],
                                 lhsT=numT[:, b * 128:(b + 1) * 128],
                                 rhs=vnum[:], start=False, stop=False)
                nc.tensor.matmul(out=psum1[:, E + 1:2 * E + 1],
                                 lhsT=numT2[:, b * 128:(b + 1) * 128],
                                 rhs=vnum2[:], start=False, stop=True)
                # FM combine for this block
                fs = smpool.tile([128, E + 1], f32, tag="fs")
                nc.vector.tensor_copy(fs[:], psum1[:, 0:E + 1])
                fsq = smpool.tile([128, E], f32, tag="fsq")
                s1 = smpool.tile([128, 1], f32, tag="s1")
                nc.vector.tensor_tensor(out=fsq[:], in0=fs[:, 0:E],
                                        in1=fs[:, 0:E], op=mybir.AluOpType.mult)
                nc.vector.tensor_reduce(out=s1[:], in_=fsq[:],
                                        axis=mybir.AxisListType.X,
                                        op=mybir.AluOpType.add)
                s2 = smpool.tile([128, 1], f32, tag="s2")
                nc.vector.tensor_reduce(out=s2[:], in_=psum1[:, E + 1:2 * E + 1],
                                        axis=mybir.AxisListType.X,
                                        op=mybir.AluOpType.add)
                d = smpool.tile([128, 1], f32, tag="d")
                nc.vector.tensor_tensor(out=d[:], in0=s1[:], in1=s2[:],
                                        op=mybir.AluOpType.subtract)
                # out = 0.5*d + fo + bias
                nc.vector.scalar_tensor_tensor(
                    out=d[:], in0=d[:], scalar=0.5, in1=fs[:, E:E + 1],
                    op0=mybir.AluOpType.mult, op1=mybir.AluOpType.add)
                nc.vector.tensor_tensor(out=outacc[:, b:b + 1], in0=d[:],
                                        in1=bias[:], op=mybir.AluOpType.add)
            nc.sync.dma_start(t_out[:], outacc[:])
    nc.compile()
    _CACHE[key] = nc
    return nc


def _host_prep_core(idxc_flat, numeric):
    """Per-core index routing + selection-matrix metadata.

    idxc_flat: [LOOKUPS] int32 (order j = local_sample*F + field)
    numeric:   [BC, NUMER] float32
    """
    j = np.arange(LOOKUPS)
    blk = j // (128 * F)                    # sample block 0..7
    m = (j // F) % 128                      # sample within block
    binv = idxc_flat >> 15                  # vocab window
    call = blk * NBIN + binv
    order = np.argsort(call, kind="stable")
    counts = np.bincount(call, minlength=NCALL).astype(np.int32)
    assert counts.max() <= CAP, f"queue overflow: {counts.max()} > {CAP}"
    starts = np.zeros(NCALL, np.int64)
    starts[1:] = np.cumsum(counts)[:-1]
    q = j - starts[call[order]]             # rank within call

    idx16 = np.full((NCALL, CAP), -1, np.int16)
    idx16[call[order], q] = (idxc_flat[order] & 32767).astype(np.int16)
    # wrapped layout: within a call, idx slot k -> [k%16, k//16]
    w16 = np.transpose(idx16.reshape(NCALL, CAP // 16, 16), (0, 2, 1))
    idxw = np.tile(w16.reshape(NCALL * 16, CAP // 16)
                   .reshape(NCALL, 16, CAP // 16)
                   .transpose(1, 0, 2).reshape(16, NCALL * (CAP // 16)), (8, 1))

    sel = np.zeros((NCALL, NCHUNK, 128, 128), np.float32)
    t_of = q // 128
    r_of = q % 128
    sel[call[order], t_of, r_of, m[order]] = 1.0
    sel8 = sel.astype(ml_dtypes.float8_e4m3fn)
    # DRAM layout [128 r, NCALL*NCHUNK*128 (call,t,m)]
    seldram = np.ascontiguousarray(
        sel8.transpose(2, 0, 1, 3).reshape(128, NCALL * NCHUNK * 128))

    numT = np.ascontiguousarray(numeric.T)
    return idxw, seldram, counts.reshape(1, NCALL), numT


def prepare_in_maps(inputs, w_one_hot, w_numeric, v_one_hot, v_numeric, b):
    inputs = np.asarray(inputs, dtype=np.float32)
    v_one_hot = np.asarray(v_one_hot, dtype=np.float32)
    w_one_hot = np.asarray(w_one_hot, dtype=np.float32)
    v_numeric = np.asarray(v_numeric, dtype=np.float32)
    w_numeric = np.asarray(w_numeric, dtype=np.float32)
    b = np.asarray(b, dtype=np.float32)

    table = np.zeros((VPAD, ROW), np.float32)
    table[:V, 0:E] = v_one_hot
    table[:V, E] = w_one_hot[:, 0]
    vnum = np.concatenate([v_numeric, w_numeric[:, 0:1]], axis=1)
    vnum = np.ascontiguousarray(vnum, dtype=np.float32)
    bias = np.tile(b.reshape(1, 1).astype(np.float32), (128, 1))

    idx_all = inputs[:, :F].astype(np.int32)
    numeric_all = inputs[:, F:]

    in_maps = []
    for c in range(NCORES):
        idxc = idx_all[c * BC:(c + 1) * BC].ravel()
        numeric = numeric_all[c * BC:(c + 1) * BC]
        idxw, seldram, counts, numT = _host_prep_core(idxc, numeric)
        in_maps.append({
            "table": table,
            "idxw": idxw,
            "sel": seldram,
            "counts": counts,
            "numT": np.ascontiguousarray(numT, dtype=np.float32),
            "vnum": vnum,
            "bias": bias,
        })
    return in_maps


def core_output_to_rows(o):
    """[128, NBLK] core output tile -> [BC] sample vector."""
    return o.T.reshape(BC)


def kernel(inputs, w_one_hot, w_numeric, v_one_hot, v_numeric, b):
    in_maps = prepare_in_maps(inputs, w_one_hot, w_numeric, v_one_hot,
                              v_numeric, b)
    nc = _build_program()
    res = run_bass_kernel_spmd(nc, in_maps, core_ids=list(range(NCORES)),
                               **_RUN_KWARGS)
    _LAST_RESULT[0] = res

    out = np.zeros((B, 1), np.float32)
    for c in range(NCORES):
        o = res.results[c]["out"]          # [128, NBLK]
        out[c * BC:(c + 1) * BC, 0] = core_output_to_rows(o)
    return out


_RUN_KWARGS = {}
_LAST_RESULT = [None]



# revision 6
# speedup vs baseline: 1.6488x; 1.6488x over previous
"""FM layer (embedding lookup) Trainium2 Bass kernel.

Strategy (data-parallel over batch, 8 cores x 1024 samples):
  - Combined table T [2^20 rows, 128 cols] fp32: cols 0:64 = v_one_hot row,
    col 64 = w_one_hot scalar. 512B rows (dma_gather needs 256B-multiple rows).
  - Host routes each core's 204800 categorical lookups into 256 queues
    (8 sample-blocks x 32 vocab windows of 32768 rows) so the int16-indexed
    dma_gather HW engine can address them (standard embedding index routing).
  - Device: per queue, one dma_gather call pulls the rows into SBUF; the
    per-(row->sample) reduction runs on the TensorEngine as a selection
    matmul (host-built 0/1 fp8 matrices, cast to fp32 on ScalarE), which
    accumulates field sums (and w sums via col 64) into PSUM per sample.
    A second matmul over ACT-squared rows accumulates the squared field sums.
  - Numeric features: small dense matmuls into the same PSUM accumulators.
  - Final FM combine (0.5*(sum fs^2 - sum fss) + first_order + bias) on DVE.
"""
import os
import numpy as np
import ml_dtypes

_SKIP = set(os.environ.get("FMK_SKIP", "").split(","))

import concourse.bass as bass
import concourse.bacc as bacc
import concourse.mybir as mybir
import concourse.tile as tile
from concourse.bass_utils import run_bass_kernel_spmd

B = 8192
F = 200            # categorical fields
NUMER = 100        # numeric features
E = 64             # embedding dim
V = 1_000_000
VPAD = 1_048_576   # 32 windows x 32768
ROW = 128          # table row elems (512B): v(64) + w(1) + pad
NCORES = 8
BC = B // NCORES   # 1024 samples per core
NBLK = BC // 128   # 8 sample blocks per core
NBIN = 32          # vocab windows (32768 rows each)
NCALL = NBLK * NBIN          # 256 gather calls per core
CAP = 1024                   # max lookups per call (avg 800, sigma 28)
NCHUNK = CAP // 128          # 9 sel-matmul chunks per call
LOOKUPS = BC * F             # 204800 per core

NQUEUES = 4                  # SWDGE queues (round-robin gather calls)
SINGLE_PACKET = True

_CACHE = {}


def _build_program(trace=False):
    key = "nc"
    if key in _CACHE:
        return _CACHE[key]
    f32 = mybir.dt.float32
    f8 = mybir.dt.float8e4
    nc = bacc.Bacc("TRN2", target_bir_lowering=False, debug=False,
                   num_devices=NCORES, num_swdge_queues=NQUEUES)
    t_table = nc.dram_tensor("table", [VPAD, ROW], f32, kind="ExternalInput")
    t_idx = nc.dram_tensor("idxw", [128, NCALL * (CAP // 16)], mybir.dt.int16,
                           kind="ExternalInput")
    t_sel = nc.dram_tensor("sel", [128, NCALL * NCHUNK * 128], f8,
                           kind="ExternalInput")
    t_cnt = nc.dram_tensor("counts", [1, NCALL], mybir.dt.int32,
                           kind="ExternalInput")
    t_numT = nc.dram_tensor("numT", [NUMER, BC], f32, kind="ExternalInput")
    t_vnum = nc.dram_tensor("vnum", [NUMER, E + 1], f32, kind="ExternalInput")
    t_bias = nc.dram_tensor("bias", [128, 1], f32, kind="ExternalInput")
    t_out = nc.dram_tensor("out", [128, NBLK], f32, kind="ExternalOutput")

    cw = CAP // 16  # idx cols per call

    with tile.TileContext(nc) as tc:
        with (
            tc.tile_pool(name="const", bufs=1) as cpool,
            tc.tile_pool(name="gp", bufs=3) as gpool,
            tc.tile_pool(name="rhs", bufs=3) as rhspool,
            tc.tile_pool(name="sel8", bufs=3) as sel8pool,
            tc.tile_pool(name="sel32", bufs=3) as sel32pool,
            tc.tile_pool(name="ps", bufs=2, space="PSUM") as pspool,
            tc.tile_pool(name="small", bufs=2) as smpool,
        ):
            idx_t = cpool.tile([128, NCALL * cw], mybir.dt.int16)
            nc.sync.dma_start(idx_t[:], t_idx[:])
            cnt_t = cpool.tile([1, NCALL], mybir.dt.int32)
            nc.sync.dma_start(cnt_t[:], t_cnt[:])
            numT = cpool.tile([NUMER, BC], f32)
            nc.sync.dma_start(numT[:], t_numT[:])
            vnum = cpool.tile([NUMER, E + 1], f32)
            nc.sync.dma_start(vnum[:], t_vnum[:])
            bias = cpool.tile([128, 1], f32)
            nc.sync.dma_start(bias[:], t_bias[:])

            numT2 = cpool.tile([NUMER, BC], f32)
            vnum2 = cpool.tile([NUMER, E], f32)
            nc.scalar.square(numT2[:], numT[:])
            nc.scalar.square(vnum2[:], vnum[:, 0:E])

            outacc = cpool.tile([128, NBLK], f32)

            cnt_reg = nc.gpsimd.alloc_register("cntreg")

            for b in range(NBLK):
                psum1 = pspool.tile([128, 2 * E + 1], f32, tag="ps1")
                for s in range(NBIN):
                    call = b * NBIN + s
                    g = gpool.tile([128, CAP // 128 * ROW], f32, tag="g")
                    g3 = g[:].rearrange("p (c d) -> p c d", d=ROW)
                    if call < 3:
                        # pad slots are skipped by the gather; on the first
                        # use of each buffer slot, stale NaNs would poison
                        # 0*NaN in the matmul. later calls see old (finite)
                        # gathered data there, which the zero sel rows kill.
                        nc.vector.memset(g[:], 0.0)
                    nc.gpsimd.reg_load(cnt_reg, cnt_t[0:1, call:call + 1])
                    nc.gpsimd.dma_gather(
                        out_ap=g3,
                        in_ap=t_table[s * 32768:(s + 1) * 32768, :],
                        idxs_ap=idx_t[:, call * cw:(call + 1) * cw],
                        num_idxs=CAP,
                        num_idxs_reg=cnt_reg,
                        elem_size=ROW,
                        single_packet=SINGLE_PACKET,
                        queue_num=call % NQUEUES,
                    )
                    sel8 = sel8pool.tile([128, NCHUNK * 128], f8, tag="sel8")
                    nc.sync.dma_start(
                        sel8[:], t_sel[:, call * NCHUNK * 128:(call + 1) * NCHUNK * 128])
                    sel32 = sel32pool.tile([128, NCHUNK * 128], f32, tag="sel32")
                    nc.scalar.copy(sel32[:], sel8[:])
                    # build rhs [v(64) | w(1) | v^2(64)] = 129 cols per row
                    rhs = rhspool.tile([128, NCHUNK * (2 * E + 1)], f32, tag="rhs")
                    rhs3 = rhs[:].rearrange("p (c d) -> p c d", d=2 * E + 1)
                    nc.vector.tensor_copy(rhs3[:, :, 0:E + 1], g3[:, :, 0:E + 1])
                    nc.scalar.square(rhs3[:, :, E + 1:2 * E + 1], g3[:, :, 0:E])
                    for t in range(NCHUNK):
                        lhsT = sel32[:, t * 128:(t + 1) * 128]
                        nc.tensor.matmul(out=psum1[:], lhsT=lhsT,
                                         rhs=rhs3[:, t, :],
                                         start=(s == 0 and t == 0), stop=False)
                # numeric contribution accumulates on top
                nc.tensor.matmul(out=psum1[:, 0:E + 1],
                                 lhsT=numT[:, b * 128:(b + 1) * 128],
                                 rhs=vnum[:], start=False, stop=False)
                nc.tensor.matmul(out=psum1[:, E + 1:2 * E + 1],
                                 lhsT=numT2[:, b * 128:(b + 1) * 128],
                                 rhs=vnum2[:], start=False, stop=True)
                # FM combine for this block
                fs = smpool.tile([128, E + 1], f32, tag="fs")
                nc.vector.tensor_copy(fs[:], psum1[:, 0:E + 1])
                fsq = smpool.tile([128, E], f32, tag="fsq")
                s1 = smpool.tile([128, 1], f32, tag="s1")
                nc.vector.tensor_tensor(out=fsq[:], in0=fs[:, 0:E],
                                        in1=fs[:, 0:E], op=mybir.AluOpType.mult)
                nc.vector.tensor_reduce(out=s1[:], in_=fsq[:],
                                        axis=mybir.AxisListType.X,
                                        op=mybir.AluOpType.add)
                s2 = smpool.tile([128, 1], f32, tag="s2")
                nc.vector.tensor_reduce(out=s2[:], in_=psum1[:, E + 1:2 * E + 1],
                                        axis=mybir.AxisListType.X,
                                        op=mybir.AluOpType.add)
                d = smpool.tile([128, 1], f32, tag="d")
                nc.vector.tensor_tensor(out=d[:], in0=s1[:], in1=s2[:],
                                        op=mybir.AluOpType.subtract)
                # out = 0.5*d + fo + bias
                nc.vector.scalar_tensor_tensor(
                    out=d[:], in0=d[:], scalar=0.5, in1=fs[:, E:E + 1],
                    op0=mybir.AluOpType.mult, op1=mybir.AluOpType.add)
                nc.vector.tensor_tensor(out=outacc[:, b:b + 1], in0=d[:],
                                        in1=bias[:], op=mybir.AluOpType.add)
            nc.sync.dma_start(t_out[:], outacc[:])
    nc.compile()
    _CACHE[key] = nc
    return nc


def _host_prep_core(idxc_flat, numeric):
    """Per-core index routing + selection-matrix metadata.

    idxc_flat: [LOOKUPS] int32 (order j = local_sample*F + field)
    numeric:   [BC, NUMER] float32
    """
    j = np.arange(LOOKUPS)
    blk = j // (128 * F)                    # sample block 0..7
    m = (j // F) % 128                      # sample within block
    binv = idxc_flat >> 15                  # vocab window
    call = blk * NBIN + binv
    order = np.argsort(call, kind="stable")
    counts = np.bincount(call, minlength=NCALL).astype(np.int32)
    assert counts.max() <= CAP, f"queue overflow: {counts.max()} > {CAP}"
    starts = np.zeros(NCALL, np.int64)
    starts[1:] = np.cumsum(counts)[:-1]
    q = j - starts[call[order]]             # rank within call

    idx16 = np.full((NCALL, CAP), -1, np.int16)
    idx16[call[order], q] = (idxc_flat[order] & 32767).astype(np.int16)
    # wrapped layout: within a call, idx slot k -> [k%16, k//16]
    w16 = np.transpose(idx16.reshape(NCALL, CAP // 16, 16), (0, 2, 1))
    idxw = np.tile(w16.reshape(NCALL * 16, CAP // 16)
                   .reshape(NCALL, 16, CAP // 16)
                   .transpose(1, 0, 2).reshape(16, NCALL * (CAP // 16)), (8, 1))

    sel = np.zeros((NCALL, NCHUNK, 128, 128), np.float32)
    t_of = q // 128
    r_of = q % 128
    sel[call[order], t_of, r_of, m[order]] = 1.0
    sel8 = sel.astype(ml_dtypes.float8_e4m3fn)
    # DRAM layout [128 r, NCALL*NCHUNK*128 (call,t,m)]
    seldram = np.ascontiguousarray(
        sel8.transpose(2, 0, 1, 3).reshape(128, NCALL * NCHUNK * 128))

    numT = np.ascontiguousarray(numeric.T)
    return idxw, seldram, counts.reshape(1, NCALL), numT


def prepare_in_maps(inputs, w_one_hot, w_numeric, v_one_hot, v_numeric, b):
    inputs = np.asarray(inputs, dtype=np.float32)
    v_one_hot = np.asarray(v_one_hot, dtype=np.float32)
    w_one_hot = np.asarray(w_one_hot, dtype=np.float32)
    v_numeric = np.asarray(v_numeric, dtype=np.float32)
    w_numeric = np.asarray(w_numeric, dtype=np.float32)
    b = np.asarray(b, dtype=np.float32)

    table = np.zeros((VPAD, ROW), np.float32)
    table[:V, 0:E] = v_one_hot
    table[:V, E] = w_one_hot[:, 0]
    vnum = np.concatenate([v_numeric, w_numeric[:, 0:1]], axis=1)
    vnum = np.ascontiguousarray(vnum, dtype=np.float32)
    bias = np.tile(b.reshape(1, 1).astype(np.float32), (128, 1))

    idx_all = inputs[:, :F].astype(np.int32)
    numeric_all = inputs[:, F:]

    in_maps = []
    for c in range(NCORES):
        idxc = idx_all[c * BC:(c + 1) * BC].ravel()
        numeric = numeric_all[c * BC:(c + 1) * BC]
        idxw, seldram, counts, numT = _host_prep_core(idxc, numeric)
        in_maps.append({
            "table": table,
            "idxw": idxw,
            "sel": seldram,
            "counts": counts,
            "numT": np.ascontiguousarray(numT, dtype=np.float32),
            "vnum": vnum,
            "bias": bias,
        })
    return in_maps


def core_output_to_rows(o):
    """[128, NBLK] core output tile -> [BC] sample vector."""
    return o.T.reshape(BC)


def kernel(inputs, w_one_hot, w_numeric, v_one_hot, v_numeric, b):
    in_maps = prepare_in_maps(inputs, w_one_hot, w_numeric, v_one_hot,
                              v_numeric, b)
    nc = _build_program()
    res = run_bass_kernel_spmd(nc, in_maps, core_ids=list(range(NCORES)),
                               **_RUN_KWARGS)
    _LAST_RESULT[0] = res

    out = np.zeros((B, 1), np.float32)
    for c in range(NCORES):
        o = res.results[c]["out"]          # [128, NBLK]
        out[c * BC:(c + 1) * BC, 0] = core_output_to_rows(o)
    return out


_RUN_KWARGS = {}
_LAST_RESULT = [None]

